# revision 22
# baseline (speedup 1.0000x reference)
"""MoE head kernel for Trainium2 (8 NeuronCores, data-parallel over batch).

Computes, per the reference nn.Module:
  w      = softmax(cos_sim(z_cat, mu_cat) / tau)          # gate  [B, E]
  xhat   = LayerNorm(feat)
  h_e    = relu(xhat @ W1'_e + b1'_e)     (affine folded: W1' = gamma*W1,
                                           b1' = b1 + beta @ W1)
  l_e    = h_e @ W2_e + b2_e
  logits = sum_e w[:, e] * l_e                             # [B, C]
returns (logits, w).

Sharding: batch B=16384 split 8 ways (2048 rows/core); params replicated.

Per-core structure:
  - LN in [B, D] layout; xhat transposed to xhatT [D, B] (bf16) via PE
    transposes, emitted one super-group after each tile's LN chain so the
    PE never waits on ACT/DVE.  (DVE StreamTranspose variant exists behind
    DVE_TRANSPOSE but hangs real HW with 3D strided APs - keep False.)
  - mm1 (bf16): loop (expert, chunk, m); per-m W1 strips resident in SBUF
    (separate tiles so group m only waits on strip m's DMA).
  - mm2 (bf16) accumulates over m into ps2[e,c] PSUM; emitted in batches
    of 4, newest-relu-first, so the tile framework's redundant-wait
    elision leaves one semaphore wait per batch (waited PE instructions
    cost ~95ns at dispatch).
  - Gate + drains in transposed space: logitsT[c,b] accumulated in SBUF;
    per (e,c): selector-matmul broadcasts wT row e to 8 partitions, then
    two DVE ops do the gate-weighted accumulate. b2 is folded with a
    single K=8 matmul of b2.T-ish against wT. No per-expert PE
    transposes; logits leave in [C, B] layout and the host transposes.
  - LN tiles 4-15 and the gate phase are interleaved into the mm1 group
    stream.
"""

import numpy as np
from contextlib import ExitStack

import ml_dtypes

import concourse.bass as bass
import concourse.mybir as mybir
import concourse.tile as tile
from concourse import bacc
from concourse.masks import make_identity
from concourse.bass_utils import run_bass_kernel_spmd

# Problem shapes (hardcoded per contract).
B, D, H, E, DZ = 16384, 1024, 2048, 8, 256
NCORES = 8
BS = B // NCORES            # rows per core = 2048
CHUNK = 512                 # batch chunk for matmul free dim
NCH = BS // CHUNK           # 4
BT = BS // 128              # 16 partition tiles of batch
KD = D // 128               # 8 K-tiles for mm1
MH = H // 128               # 16 M-tiles of hidden
KZ = DZ // 128              # 2 K-tiles for the gate matmul
LN_EPS = 1e-5

F32 = mybir.dt.float32
BF16 = mybir.dt.bfloat16
AF = mybir.ActivationFunctionType
ALU = mybir.AluOpType
AX = mybir.AxisListType

DVE_TRANSPOSE = False       # LN transposes on DVE instead of PE


def _build(tau: float):
    nc = bacc.Bacc(None, target_bir_lowering=False, name="moe_head")

    feat = nc.dram_tensor("feat", [BS, D], F32, kind="ExternalInput")
    z = nc.dram_tensor("z", [BS, DZ], F32, kind="ExternalInput")
    mu = nc.dram_tensor("mu", [E, DZ], F32, kind="ExternalInput")
    # w1p: [E, ki, MH, KD, mi] -> per-m strip DMA is contiguous per partition.
    w1p = nc.dram_tensor("w1p", [E, 128, MH, KD, 128], BF16,
                         kind="ExternalInput")
    b1p = nc.dram_tensor("b1p", [E, 128, MH], F32, kind="ExternalInput")
    w2p = nc.dram_tensor("w2p", [E, 128, MH, E], BF16, kind="ExternalInput")
    b2r = nc.dram_tensor("b2r", [E, E], BF16, kind="ExternalInput")  # [e, c]
    selp = nc.dram_tensor("selp", [E, E, E], BF16, kind="ExternalInput")
    logits_o = nc.dram_tensor("logits", [E, BS], F32, kind="ExternalOutput")
    w_o = nc.dram_tensor("w", [128, BT * E], F32, kind="ExternalOutput")

    inv_tau = 1.0 / tau

    with tile.TileContext(nc) as tc, ExitStack() as ctx:
        persist = ctx.enter_context(tc.tile_pool(name="persist", bufs=1))
        lnpool = ctx.enter_context(tc.tile_pool(name="ln", bufs=2))
        statp = ctx.enter_context(tc.tile_pool(name="stat", bufs=4))
        wpool = ctx.enter_context(tc.tile_pool(name="w1s", bufs=2))
        epool = ctx.enter_context(tc.tile_pool(name="eparam", bufs=2))
        hpool = ctx.enter_context(tc.tile_pool(name="h", bufs=8))
        spool = ctx.enter_context(tc.tile_pool(name="small", bufs=3))
        psA = ctx.enter_context(tc.tile_pool(name="psA", bufs=1, space="PSUM"))
        psB = ctx.enter_context(tc.tile_pool(name="psB", bufs=2, space="PSUM"))
        psC = ctx.enter_context(tc.tile_pool(name="psC", bufs=2, space="PSUM"))
        ztp = ctx.enter_context(tc.tile_pool(name="ztp", bufs=5))
        ftp = ctx.enter_context(tc.tile_pool(name="ftp", bufs=4))
        znp = ctx.enter_context(tc.tile_pool(name="znp", bufs=7))
        xhp = ctx.enter_context(tc.tile_pool(name="xhp", bufs=3))
        w8p = ctx.enter_context(tc.tile_pool(name="w8p", bufs=2))

        # Persistent SBUF tensors.
        xhatT_c = [persist.tile([128, KD, CHUNK], BF16, name=f"xhatT{c}")
                   for c in range(NCH)]
        znT = persist.tile([128, KZ, BS], BF16)
        munT = persist.tile([128, KZ, E], BF16)
        w_sb = persist.tile([128, BT, E], F32)        # gate weights [B, E]
        wT = persist.tile([E, BS], BF16)              # gate weights [E, B]
        lT = persist.tile([E, BS], F32)               # logitsT accum [C, B]
        sel_sb = persist.tile([E, E, E], BF16)        # selector [i, e, c]
        b2sb = persist.tile([E, E], BF16)
        ident = persist.tile([128, 128], F32)
        identB = persist.tile([128, 128], BF16)
        eps_sb = persist.tile([128, 1], F32)

        make_identity(nc, ident)
        make_identity(nc, identB)
        nc.vector.memset(lT[:], 0.0)
        nc.vector.memset(eps_sb[:], LN_EPS)
        nc.sync.dma_start(sel_sb[:], selp[:, :, :])
        nc.sync.dma_start(b2sb[:], b2r[:, :])

        # ---------- emission helpers ----------
        xh_tiles = {}
        zn_tiles = {}

        def emit_ln_tile(bt, tp_inline=True):
            bsl = slice(bt * 128, (bt + 1) * 128)
            ft = ftp.tile([128, D], F32, tag="ft")
            nc.sync.dma_start(ft[:], feat[bsl, :])
            sq = lnpool.tile([128, D], F32, tag="sq")
            s1 = statp.tile([128, 1], F32, tag="s1")
            if bt < 4:          # startup: ScalarE is idle, DVE is critical
                nc.scalar.activation(sq, ft[:], AF.Identity, accum_out=s1)
            else:
                nc.vector.reduce_sum(s1, ft[:], axis=AX.X)
            nm = statp.tile([128, 1], F32, tag="nm")
            nc.vector.tensor_scalar_mul(nm, s1, -1.0 / D)
            xc = lnpool.tile([128, D], F32, tag="xc")
            if bt < 4:
                nc.scalar.activation(xc[:], ft[:], AF.Identity, bias=nm)
            else:
                nc.vector.tensor_scalar_add(xc[:], ft[:], nm)
            ss = statp.tile([128, 1], F32, tag="ss")
            nc.scalar.activation(sq, xc[:], AF.Square, accum_out=ss)
            std = statp.tile([128, 1], F32, tag="std")
            nc.scalar.activation(std, ss, AF.Sqrt, bias=eps_sb[:],
                                 scale=1.0 / D)
            rs = statp.tile([128, 1], F32, tag="rs")
            nc.vector.reciprocal(rs, std)
            emit_ln_chain_tail(bt, xc, rs)
            if tp_inline:
                emit_ln_tp(bt)

        def emit_ln_chain_tail(bt, xc, rs):
            c, lo = divmod(bt * 128, CHUNK)
            if DVE_TRANSPOSE:
                xhb = lnpool.tile([128, KD, 128], BF16, tag="xhb")
                for kd in range(KD):
                    nc.vector.tensor_scalar_mul(
                        xhb[:, kd, :], xc[:, kd * 128:(kd + 1) * 128], rs)
                for i in range(4):
                    for j in range(4):
                        nc.vector.transpose(
                            xhatT_c[c][32 * j:32 * j + 32, :,
                                       lo + 32 * i:lo + 32 * i + 32],
                            xhb[32 * i:32 * i + 32, :, 32 * j:32 * j + 32])
            else:
                xhb = xhp.tile([128, KD, 128], BF16, tag="xh",
                               name=f"xh{bt}")
                for kd in range(KD):
                    if bt < 4:
                        nc.scalar.activation(
                            xhb[:, kd, :], xc[:, kd * 128:(kd + 1) * 128],
                            AF.Identity, scale=rs)
                    else:
                        nc.vector.tensor_scalar_mul(
                            xhb[:, kd, :], xc[:, kd * 128:(kd + 1) * 128], rs)
                xh_tiles[bt] = xhb

        def emit_ln_tp(bt):
            if DVE_TRANSPOSE:
                return
            c, lo = divmod(bt * 128, CHUNK)
            xhb = xh_tiles.pop(bt)
            for kd in range(KD):
                pst = psC.tile([128, 128], BF16, tag="tp")
                nc.tensor.transpose(
                    pst[:], xhb[:, kd, :], identB[:])
                nc.vector.tensor_copy(
                    xhatT_c[c][:, kd, lo:lo + 128], pst[:])

        def emit_gate_mu():
            mu_sb = spool.tile([E, DZ], F32, tag="mu")
            nc.sync.dma_start(mu_sb[:], mu[:, :])
            musq = spool.tile([E, DZ], F32, tag="musq")
            muss = statp.tile([E, 1], F32, tag="muss")
            nc.scalar.activation(musq, mu_sb, AF.Square, accum_out=muss)
            mustd = statp.tile([E, 1], F32, tag="mustd")
            nc.scalar.activation(mustd, muss, AF.Sqrt)
            murn = statp.tile([E, 1], F32, tag="murn")
            nc.vector.reciprocal(murn, mustd)
            mu_n = spool.tile([E, DZ], F32, tag="mun")
            nc.vector.tensor_scalar_mul(mu_n[:], mu_sb[:], murn)
            for kz in range(KZ):
                pst = psC.tile([128, 128], F32, tag="tp")
                nc.tensor.transpose(
                    pst[:, :E], mu_n[:, kz * 128:(kz + 1) * 128], ident[:E, :E])
                nc.vector.tensor_copy(munT[:, kz, :], pst[:, :E])

        z_tiles = {}

        def emit_z_dma(bt):
            bsl = slice(bt * 128, (bt + 1) * 128)
            zt = ztp.tile([128, DZ], F32, tag="zt", name=f"zt{bt}")
            nc.sync.dma_start(zt[:], z[bsl, :])
            z_tiles[bt] = zt

        def emit_gate_z(bt):
            bsl = slice(bt * 128, (bt + 1) * 128)
            zt = z_tiles.pop(bt)
            zsq = lnpool.tile([128, DZ], F32, tag="zsq")
            zss = statp.tile([128, 1], F32, tag="zss")
            nc.scalar.activation(zsq, zt, AF.Square, accum_out=zss)
            zstd = statp.tile([128, 1], F32, tag="zstd")
            nc.scalar.activation(zstd, zss, AF.Sqrt)
            zrn = statp.tile([128, 1], F32, tag="zrn")
            nc.vector.reciprocal(zrn, zstd)
            zn = znp.tile([128, DZ], F32, tag="zn", name=f"zn{bt}")
            nc.scalar.activation(zn[:], zt[:], AF.Identity, scale=zrn)
            zn_tiles[bt] = zn

        def emit_z_tp(bt):
            bsl = slice(bt * 128, (bt + 1) * 128)
            zn = zn_tiles.pop(bt)
            for kz in range(KZ):
                pst = psC.tile([128, 128], F32, tag="tp")
                nc.tensor.transpose(
                    pst[:], zn[:, kz * 128:(kz + 1) * 128], ident[:])
                nc.vector.tensor_copy(znT[:, kz, bsl], pst[:])

        def emit_gate_sims(bt):
            bsl = slice(bt * 128, (bt + 1) * 128)
            ps = psC.tile([128, E], F32, tag="tp")
            for kz in range(KZ):
                nc.tensor.matmul(
                    ps[:], znT[:, kz, bsl], munT[:, kz, :],
                    start=(kz == 0), stop=(kz == KZ - 1))
            mx = statp.tile([128, 1], F32, tag="mx")
            nc.vector.reduce_max(mx, ps[:], axis=AX.X)
            nb = statp.tile([128, 1], F32, tag="nb")
            nc.vector.tensor_scalar_mul(nb, mx, -inv_tau)
            ex = spool.tile([128, E], F32, tag="ex")
            nc.scalar.activation(ex[:], ps[:], AF.Exp, bias=nb, scale=inv_tau)
            sm = statp.tile([128, 1], F32, tag="sm")
            nc.vector.reduce_sum(sm, ex[:], axis=AX.X)
            rsm = statp.tile([128, 1], F32, tag="rsm")
            nc.vector.reciprocal(rsm, sm)
            nc.vector.tensor_scalar_mul(w_sb[:, bt, :], ex[:], rsm)

        def emit_wT(bt):
            # wT[e, b] from w_sb tiles (PE transpose, fp32 -> bf16 copy).
            pst = psC.tile([E, 128], F32, tag="tp")
            nc.tensor.transpose(pst[:], w_sb[:, bt, :], ident[:])
            nc.vector.tensor_copy(wT[:, bt * 128:(bt + 1) * 128], pst[:])

        def emit_b2fold():
            # lT += (w @ b2) in [C, B] layout: one K=8 matmul per chunk.
            for c in range(NCH):
                csl = slice(c * CHUNK, (c + 1) * CHUNK)
                ps = psC.tile([E, CHUNK], F32, tag="tp")
                nc.tensor.matmul(ps[:], b2sb[:, :], wT[:, csl],
                                 start=True, stop=True)
                nc.vector.tensor_tensor(lT[:, csl], lT[:, csl], ps[:], ALU.add)

        w1_tiles = {}
        eparams = {}

        def fetch_expert(e):
            if e >= E or e in w1_tiles:
                return
            strips = []
            for m in range(MH):
                t = wpool.tile([128, KD, 128], BF16, tag=f"w1_{m}",
                               name=f"w1_{e}_{m}")
                nc.sync.dma_start(t[:], w1p[e, :, m])
                strips.append(t)
            w2sb = epool.tile([128, MH, E], BF16, tag="w2", name=f"w2_{e}")
            nc.sync.dma_start(w2sb[:], w2p[e])
            b1sb = epool.tile([128, MH], F32, tag="b1", name=f"b1_{e}")
            nc.sync.dma_start(b1sb[:], b1p[e])
            w1_tiles[e] = strips
            eparams[e] = (w2sb, b1sb)

        w8_tiles = {}

        def emit_w8(e):
            w8b = w8p.tile([E, BS], BF16, tag="w8b", name=f"w8_{e}")
            for c in range(NCH):
                csl = slice(c * CHUNK, (c + 1) * CHUNK)
                w8 = psC.tile([E, CHUNK], F32, tag="tp")
                nc.tensor.matmul(w8[:], sel_sb[:, e, :], wT[:, csl],
                                 start=True, stop=True)
                nc.vector.tensor_copy(w8b[:, csl], w8[:])
            w8_tiles[e] = w8b

        def emit_drain(e, c, ps2):
            if e not in w8_tiles:
                emit_w8(e)
            csl = slice(c * CHUNK, (c + 1) * CHUNK)
            tmp = spool.tile([E, CHUNK], F32, tag="ltmp")
            nc.vector.tensor_tensor(tmp[:], ps2[:], w8_tiles[e][:, csl],
                                    ALU.mult)
            nc.vector.tensor_tensor(lT[:, csl], lT[:, csl], tmp[:], ALU.add)

        # ---------------- Phase A: first chunk of LN ----------------
        for bt in range(4):
            emit_ln_tile(bt)

        # ---------------- Main super-group stream ----------------
        # Super-group = (e, c, m2) covering m = 2*m2, 2*m2+1.  256 total,
        # ~3.7us each.  Side work is scheduled by super-group index.
        side_work = {}
        for i in range(12):                      # LN tiles 4..15: chain,
            side_work.setdefault(i, []).append(  # then transposes 1 sg later
                lambda bt=4 + i: emit_ln_chain(bt))
            side_work.setdefault(i + 1, []).append(
                lambda bt=4 + i: emit_ln_tp(bt))
        side_work.setdefault(7, []).append(emit_gate_mu)
        for i in range(16):                      # gate z DMAs (4 sg lead)
            side_work.setdefault(8 + i // 2, []).append(
                lambda bt=i: emit_z_dma(bt))
        for i in range(16):                      # z normalize chain
            side_work.setdefault(12 + i // 2, []).append(
                lambda bt=i: emit_gate_z(bt))
        for i in range(16):                      # z transposes 3 sgs later
            side_work.setdefault(15 + i // 2, []).append(
                lambda bt=i: emit_z_tp(bt))
        for i in range(16):                      # sims + softmax, then wT
            side_work.setdefault(17 + i // 2, []).append(
                lambda bt=i: emit_gate_sims(bt))
            side_work.setdefault(18 + i // 2, []).append(
                lambda bt=i: emit_wT(bt))
        side_work.setdefault(27, []).append(emit_b2fold)
        side_work.setdefault(30, []).append(
            lambda: nc.sync.dma_start(
                w_o.rearrange("p (bo c) -> p bo c", c=E), w_sb[:]))

        def emit_ln_chain(bt):
            emit_ln_tile(bt, tp_inline=False)

        fetch_expert(0)
        sgroups = [(e, c, m2) for e in range(E) for c in range(NCH)
                   for m2 in range(MH // 2)]
        pend_h = []              # [(e, c, m, hsb), ...] awaiting mm2
        pend_drain = []          # [[countdown, e, c, ps2], ...]
        ps2_cur = None

        def flush_mm2(n, gi):
            nonlocal ps2_cur
            batch = pend_h[:n]
            del pend_h[:n]
            ms = [b[2] for b in batch]
            order = sorted(range(len(batch)), key=lambda i: -ms[i])
            if 0 in ms:                       # start must execute first
                i0 = ms.index(0)
                order.remove(i0)
                order.insert(0, i0)
            if MH - 1 in ms:                  # stop must execute last
                i15 = ms.index(MH - 1)
                order.remove(i15)
                order.append(i15)
            for i in order:
                pe, pc, pm, ph = batch[i]
                if pm == 0:
                    ps2_cur = psB.tile([E, CHUNK], F32, tag="ps2",
                                       name=f"ps2_{pe}_{pc}")
                pw2, _ = eparams[pe]
                nc.tensor.matmul(
                    ps2_cur[:], pw2[:, pm, :], ph[:],
                    start=(pm == 0), stop=(pm == MH - 1))
                if pm == MH - 1:
                    pend_drain.append([max(2, 29 - gi), pe, pc, ps2_cur])

        # Four dedicated ps1 banks.  Within a super-group the first m-half
        # writes the bank whose previous reader (relu) is NEWER, so its WAR
        # wait subsumes the second half's and the tile framework elides one
        # semaphore wait per super-group.
        SLOT = {0: (0, 1), 1: (2, 3), 2: (1, 0), 3: (3, 2)}

        def mm1_group(e, c, m, slot):
            strips = w1_tiles[e]
            _, b1sb = eparams[e]
            ps1 = psA.tile([128, CHUNK], F32, tag=f"ps1_{slot}")
            for k in range(KD):
                nc.tensor.matmul(
                    ps1[:], strips[m][:, k, :], xhatT_c[c][:, k, :],
                    start=(k == 0), stop=(k == KD - 1))
            hsb = hpool.tile([128, CHUNK], BF16, tag="h")
            nc.scalar.activation(
                hsb[:], ps1[:], AF.Relu, bias=b1sb[:, m:m + 1])
            pend_h.append((e, c, m, hsb))

        for gi, (e, c, m2) in enumerate(sgroups):
            if c == 0 and m2 == 1:
                fetch_expert(e + 1)
            sa, sb = SLOT[gi % 4]
            mm1_group(e, c, 2 * m2, sa)
            mm1_group(e, c, 2 * m2 + 1, sb)

            if len(pend_h) > 4:
                flush_mm2(4, gi)

            for item in pend_drain:
                item[0] -= 1
            while pend_drain and pend_drain[0][0] <= 0:
                _, de, dc, dps2 = pend_drain.pop(0)
                emit_drain(de, dc, dps2)

            for fn in side_work.pop(gi, ()):
                fn()

        # Tail: remaining mm2 batches + drains.
        while pend_h:
            flush_mm2(min(4, len(pend_h)), len(sgroups))
        for _, de, dc, dps2 in pend_drain:
            emit_drain(de, dc, dps2)

        # ---------------- Outputs (contiguous; host reorders) ----------
        nc.sync.dma_start(logits_o[:, :], lT[:])

    nc.compile()
    return nc


_CACHE = {}
_PREP = {}


def _prepare_params(W1, b1, W2, b2, ln_gamma, ln_beta):
    """Fold LN affine into W1/b1, convert + lay out for the kernel."""
    key = id(W1)
    if key in _PREP:
        return _PREP[key]
    if np.all(ln_gamma == 1.0):
        W1f = W1.astype(np.float32)
    else:
        W1f = (ln_gamma[:, :, None].astype(np.float64) *
               W1.astype(np.float64)).astype(np.float32)
    if np.all(ln_beta == 0.0):
        b1f = b1.astype(np.float32)
    else:
        b1f = (b1.astype(np.float64) +
               np.einsum('ed,edh->eh', ln_beta.astype(np.float64),
                         W1.astype(np.float64))).astype(np.float32)

    w1pp = np.ascontiguousarray(
        W1f.reshape(E, KD, 128, MH, 128).transpose(0, 2, 3, 1, 4)
    ).astype(ml_dtypes.bfloat16)
    w2pp = np.ascontiguousarray(
        W2.astype(np.float32).reshape(E, MH, 128, E).transpose(0, 2, 1, 3)
    ).astype(ml_dtypes.bfloat16)
    b1pp = np.ascontiguousarray(
        b1f.reshape(E, MH, 128).transpose(0, 2, 1))
    b2rp = np.ascontiguousarray(b2.astype(np.float32)).astype(
        ml_dtypes.bfloat16)
    selp = np.zeros((E, E, E), ml_dtypes.bfloat16)
    for e in range(E):
        selp[e, e, :] = 1.0
    _PREP.clear()
    _PREP[key] = (w1pp, b1pp, w2pp, b2rp, selp)
    return _PREP[key]


def kernel(**inputs):
    feat = np.ascontiguousarray(inputs["feat"], dtype=np.float32)
    z_cat = np.ascontiguousarray(inputs["z_cat"], dtype=np.float32)
    mu_cat = np.ascontiguousarray(inputs["mu_cat"], dtype=np.float32)
    ln_gamma = np.asarray(inputs["ln_gamma"], dtype=np.float32)
    ln_beta = np.asarray(inputs["ln_beta"], dtype=np.float32)
    tau = max(1e-6, float(inputs["tau_gate"]))

    w1pp, b1pp, w2pp, b2rp, selp = _prepare_params(
        np.asarray(inputs["W1"]), np.asarray(inputs["b1"]),
        np.asarray(inputs["W2"]), np.asarray(inputs["b2"]),
        ln_gamma, ln_beta)

    if tau not in _CACHE:
        _CACHE[tau] = _build(tau)
    nc = _CACHE[tau]

    in_maps = []
    for c in range(NCORES):
        rs = slice(c * BS, (c + 1) * BS)
        in_maps.append({
            "feat": feat[rs],
            "z": z_cat[rs],
            "mu": mu_cat,
            "w1p": w1pp,
            "b1p": b1pp,
            "w2p": w2pp,
            "b2r": b2rp,
            "selp": selp,
        })
    global LAST_IN_MAPS
    LAST_IN_MAPS = in_maps

    res = run_bass_kernel_spmd(nc, in_maps, core_ids=list(range(NCORES)))
    outs = res.results
    logits = np.concatenate([o["logits"].T for o in outs], axis=0)
    w = np.concatenate(
        [o["w"].reshape(128, BT, E).transpose(1, 0, 2).reshape(BS, E)
         for o in outs], axis=0)
    return logits.astype(np.float32), w.astype(np.float32)


# revision 23
# speedup vs baseline: 1.0105x; 1.0105x over previous
"""MoE head kernel for Trainium2 (8 NeuronCores, data-parallel over batch).

Computes, per the reference nn.Module:
  w      = softmax(cos_sim(z_cat, mu_cat) / tau)          # gate  [B, E]
  xhat   = LayerNorm(feat)
  h_e    = relu(xhat @ W1'_e + b1'_e)     (affine folded: W1' = gamma*W1,
                                           b1' = b1 + beta @ W1)
  l_e    = h_e @ W2_e + b2_e
  logits = sum_e w[:, e] * l_e                             # [B, C]
returns (logits, w).

Sharding: batch B=16384 split 8 ways (2048 rows/core); params replicated.

Per-core structure:
  - LN in [B, D] layout; xhat transposed to xhatT [D, B] (bf16) via PE
    transposes, emitted one super-group after each tile's LN chain so the
    PE never waits on ACT/DVE.  (DVE StreamTranspose variant exists behind
    DVE_TRANSPOSE but hangs real HW with 3D strided APs - keep False.)
  - mm1 (bf16): loop (expert, chunk, m); per-m W1 strips resident in SBUF
    (separate tiles so group m only waits on strip m's DMA).
  - mm2 (bf16) accumulates over m into ps2[e,c] PSUM; emitted in batches
    of 4, newest-relu-first, so the tile framework's redundant-wait
    elision leaves one semaphore wait per batch (waited PE instructions
    cost ~95ns at dispatch).
  - Gate + drains in transposed space: logitsT[c,b] accumulated in SBUF;
    per (e,c): selector-matmul broadcasts wT row e to 8 partitions, then
    two DVE ops do the gate-weighted accumulate. b2 is folded with a
    single K=8 matmul of b2.T-ish against wT. No per-expert PE
    transposes; logits leave in [C, B] layout and the host transposes.
  - LN tiles 4-15 and the gate phase are interleaved into the mm1 group
    stream.
"""

import numpy as np
from contextlib import ExitStack

import ml_dtypes

import concourse.bass as bass
import concourse.mybir as mybir
import concourse.tile as tile
from concourse import bacc
from concourse.masks import make_identity
from concourse.bass_utils import run_bass_kernel_spmd

# Problem shapes (hardcoded per contract).
B, D, H, E, DZ = 16384, 1024, 2048, 8, 256
NCORES = 8
BS = B // NCORES            # rows per core = 2048
CHUNK = 512                 # batch chunk for matmul free dim
NCH = BS // CHUNK           # 4
BT = BS // 128              # 16 partition tiles of batch
KD = D // 128               # 8 K-tiles for mm1
MH = H // 128               # 16 M-tiles of hidden
KZ = DZ // 128              # 2 K-tiles for the gate matmul
LN_EPS = 1e-5

F32 = mybir.dt.float32
BF16 = mybir.dt.bfloat16
AF = mybir.ActivationFunctionType
ALU = mybir.AluOpType
AX = mybir.AxisListType

DVE_TRANSPOSE = False       # LN transposes on DVE instead of PE


def _build(tau: float):
    nc = bacc.Bacc(None, target_bir_lowering=False, name="moe_head")

    feat = nc.dram_tensor("feat", [BS, D], F32, kind="ExternalInput")
    z = nc.dram_tensor("z", [BS, DZ], F32, kind="ExternalInput")
    mu = nc.dram_tensor("mu", [E, DZ], F32, kind="ExternalInput")
    # w1p: [E, ki, MH, KD, mi] -> per-m strip DMA is contiguous per partition.
    w1p = nc.dram_tensor("w1p", [E, 128, MH, KD, 128], BF16,
                         kind="ExternalInput")
    b1p = nc.dram_tensor("b1p", [E, 128, MH], F32, kind="ExternalInput")
    w2p = nc.dram_tensor("w2p", [E, 128, MH, E], BF16, kind="ExternalInput")
    b2r = nc.dram_tensor("b2r", [E, E], BF16, kind="ExternalInput")  # [e, c]
    selp = nc.dram_tensor("selp", [E, E, E], BF16, kind="ExternalInput")
    logits_o = nc.dram_tensor("logits", [E, BS], F32, kind="ExternalOutput")
    w_o = nc.dram_tensor("w", [128, BT * E], F32, kind="ExternalOutput")

    inv_tau = 1.0 / tau

    with tile.TileContext(nc) as tc, ExitStack() as ctx:
        persist = ctx.enter_context(tc.tile_pool(name="persist", bufs=1))
        lnpool = ctx.enter_context(tc.tile_pool(name="ln", bufs=2))
        statp = ctx.enter_context(tc.tile_pool(name="stat", bufs=4))
        wpool = ctx.enter_context(tc.tile_pool(name="w1s", bufs=2))
        epool = ctx.enter_context(tc.tile_pool(name="eparam", bufs=2))
        hpool = ctx.enter_context(tc.tile_pool(name="h", bufs=10))
        spool = ctx.enter_context(tc.tile_pool(name="small", bufs=3))
        psA = ctx.enter_context(tc.tile_pool(name="psA", bufs=1, space="PSUM"))
        psB = ctx.enter_context(tc.tile_pool(name="psB", bufs=2, space="PSUM"))
        psC = ctx.enter_context(tc.tile_pool(name="psC", bufs=2, space="PSUM"))
        ztp = ctx.enter_context(tc.tile_pool(name="ztp", bufs=6))
        ftp = ctx.enter_context(tc.tile_pool(name="ftp", bufs=4))
        znp = ctx.enter_context(tc.tile_pool(name="znp", bufs=3))
        xhp = ctx.enter_context(tc.tile_pool(name="xhp", bufs=3))
        w8p = ctx.enter_context(tc.tile_pool(name="w8p", bufs=2))

        # Persistent SBUF tensors.
        xhatT_c = [persist.tile([128, KD, CHUNK], BF16, name=f"xhatT{c}")
                   for c in range(NCH)]
        znT = persist.tile([128, KZ, BS], BF16)
        munT = persist.tile([128, KZ, E], BF16)
        w_sb = persist.tile([128, BT, E], F32)        # gate weights [B, E]
        wT = persist.tile([E, BS], BF16)              # gate weights [E, B]
        lT = persist.tile([E, BS], F32)               # logitsT accum [C, B]
        sel_sb = persist.tile([E, E, E], BF16)        # selector [i, e, c]
        b2sb = persist.tile([E, E], BF16)
        ident = persist.tile([128, 128], F32)
        identB = persist.tile([128, 128], BF16)
        eps_sb = persist.tile([128, 1], F32)

        make_identity(nc, ident)
        make_identity(nc, identB)
        nc.vector.memset(lT[:], 0.0)
        nc.vector.memset(eps_sb[:], LN_EPS)
        nc.sync.dma_start(sel_sb[:], selp[:, :, :])
        nc.sync.dma_start(b2sb[:], b2r[:, :])

        # ---------- emission helpers ----------
        xh_tiles = {}
        zn_tiles = {}

        def emit_ln_tile(bt, tp_inline=True):
            bsl = slice(bt * 128, (bt + 1) * 128)
            ft = ftp.tile([128, D], F32, tag="ft")
            nc.sync.dma_start(ft[:], feat[bsl, :])
            sq = lnpool.tile([128, D], F32, tag="sq")
            s1 = statp.tile([128, 1], F32, tag="s1")
            if bt < 4:          # startup: ScalarE is idle, DVE is critical
                nc.scalar.activation(sq, ft[:], AF.Identity, accum_out=s1)
            else:
                nc.vector.reduce_sum(s1, ft[:], axis=AX.X)
            nm = statp.tile([128, 1], F32, tag="nm")
            nc.vector.tensor_scalar_mul(nm, s1, -1.0 / D)
            xc = lnpool.tile([128, D], F32, tag="xc")
            if bt < 4:
                nc.scalar.activation(xc[:], ft[:], AF.Identity, bias=nm)
            else:
                nc.vector.tensor_scalar_add(xc[:], ft[:], nm)
            ss = statp.tile([128, 1], F32, tag="ss")
            nc.scalar.activation(sq, xc[:], AF.Square, accum_out=ss)
            std = statp.tile([128, 1], F32, tag="std")
            nc.scalar.activation(std, ss, AF.Sqrt, bias=eps_sb[:],
                                 scale=1.0 / D)
            rs = statp.tile([128, 1], F32, tag="rs")
            nc.vector.reciprocal(rs, std)
            emit_ln_chain_tail(bt, xc, rs)
            if tp_inline:
                emit_ln_tp(bt)

        def emit_ln_chain_tail(bt, xc, rs):
            c, lo = divmod(bt * 128, CHUNK)
            if DVE_TRANSPOSE:
                xhb = lnpool.tile([128, KD, 128], BF16, tag="xhb")
                for kd in range(KD):
                    nc.vector.tensor_scalar_mul(
                        xhb[:, kd, :], xc[:, kd * 128:(kd + 1) * 128], rs)
                for i in range(4):
                    for j in range(4):
                        nc.vector.transpose(
                            xhatT_c[c][32 * j:32 * j + 32, :,
                                       lo + 32 * i:lo + 32 * i + 32],
                            xhb[32 * i:32 * i + 32, :, 32 * j:32 * j + 32])
            else:
                xhb = xhp.tile([128, KD, 128], BF16, tag="xh",
                               name=f"xh{bt}")
                for kd in range(KD):
                    nc.vector.tensor_scalar_mul(
                        xhb[:, kd, :], xc[:, kd * 128:(kd + 1) * 128], rs)
                xh_tiles[bt] = xhb

        def emit_ln_tp(bt):
            if DVE_TRANSPOSE:
                return
            c, lo = divmod(bt * 128, CHUNK)
            xhb = xh_tiles.pop(bt)
            for kd in range(KD):
                pst = psC.tile([128, 128], BF16, tag="tp")
                nc.tensor.transpose(
                    pst[:], xhb[:, kd, :], identB[:])
                nc.vector.tensor_copy(
                    xhatT_c[c][:, kd, lo:lo + 128], pst[:])

        def emit_gate_mu():
            mu_sb = spool.tile([E, DZ], F32, tag="mu")
            nc.sync.dma_start(mu_sb[:], mu[:, :])
            musq = spool.tile([E, DZ], F32, tag="musq")
            muss = statp.tile([E, 1], F32, tag="muss")
            nc.scalar.activation(musq, mu_sb, AF.Square, accum_out=muss)
            mustd = statp.tile([E, 1], F32, tag="mustd")
            nc.scalar.activation(mustd, muss, AF.Sqrt)
            murn = statp.tile([E, 1], F32, tag="murn")
            nc.vector.reciprocal(murn, mustd)
            mu_n = spool.tile([E, DZ], F32, tag="mun")
            nc.vector.tensor_scalar_mul(mu_n[:], mu_sb[:], murn)
            for kz in range(KZ):
                pst = psC.tile([128, 128], F32, tag="tp")
                nc.tensor.transpose(
                    pst[:, :E], mu_n[:, kz * 128:(kz + 1) * 128], ident[:E, :E])
                nc.vector.tensor_copy(munT[:, kz, :], pst[:, :E])

        z_tiles = {}

        def emit_z_dma(bt):
            bsl = slice(bt * 128, (bt + 1) * 128)
            zt = ztp.tile([128, DZ], F32, tag="zt", name=f"zt{bt}")
            nc.sync.dma_start(zt[:], z[bsl, :])
            z_tiles[bt] = zt

        def emit_gate_z(bt):
            bsl = slice(bt * 128, (bt + 1) * 128)
            zt = z_tiles.pop(bt)
            zsq = lnpool.tile([128, DZ], F32, tag="zsq")
            zss = statp.tile([128, 1], F32, tag="zss")
            nc.scalar.activation(zsq, zt, AF.Square, accum_out=zss)
            zstd = statp.tile([128, 1], F32, tag="zstd")
            nc.scalar.activation(zstd, zss, AF.Sqrt)
            zrn = statp.tile([128, 1], F32, tag="zrn")
            nc.vector.reciprocal(zrn, zstd)
            zn = znp.tile([128, DZ], F32, tag="zn", name=f"zn{bt}")
            nc.scalar.activation(zn[:], zt[:], AF.Identity, scale=zrn)
            zn_tiles[bt] = zn

        def emit_z_tp(bt):
            bsl = slice(bt * 128, (bt + 1) * 128)
            zn = zn_tiles.pop(bt)
            for kz in range(KZ):
                pst = psC.tile([128, 128], F32, tag="tp")
                nc.tensor.transpose(
                    pst[:], zn[:, kz * 128:(kz + 1) * 128], ident[:])
                nc.vector.tensor_copy(znT[:, kz, bsl], pst[:])

        def emit_gate_sims(bt):
            bsl = slice(bt * 128, (bt + 1) * 128)
            ps = psC.tile([128, E], F32, tag="tp")
            for kz in range(KZ):
                nc.tensor.matmul(
                    ps[:], znT[:, kz, bsl], munT[:, kz, :],
                    start=(kz == 0), stop=(kz == KZ - 1))
            mx = statp.tile([128, 1], F32, tag="mx")
            nc.vector.reduce_max(mx, ps[:], axis=AX.X)
            nb = statp.tile([128, 1], F32, tag="nb")
            nc.vector.tensor_scalar_mul(nb, mx, -inv_tau)
            ex = spool.tile([128, E], F32, tag="ex")
            nc.scalar.activation(ex[:], ps[:], AF.Exp, bias=nb, scale=inv_tau)
            sm = statp.tile([128, 1], F32, tag="sm")
            nc.vector.reduce_sum(sm, ex[:], axis=AX.X)
            rsm = statp.tile([128, 1], F32, tag="rsm")
            nc.vector.reciprocal(rsm, sm)
            nc.vector.tensor_scalar_mul(w_sb[:, bt, :], ex[:], rsm)

        def emit_wT(bt):
            # wT[e, b] from w_sb tiles (PE transpose, fp32 -> bf16 copy).
            pst = psC.tile([E, 128], F32, tag="tp")
            nc.tensor.transpose(pst[:], w_sb[:, bt, :], ident[:])
            nc.vector.tensor_copy(wT[:, bt * 128:(bt + 1) * 128], pst[:])

        def emit_b2fold():
            # lT += (w @ b2) in [C, B] layout: one K=8 matmul per chunk.
            for c in range(NCH):
                csl = slice(c * CHUNK, (c + 1) * CHUNK)
                ps = psC.tile([E, CHUNK], F32, tag="tp")
                nc.tensor.matmul(ps[:], b2sb[:, :], wT[:, csl],
                                 start=True, stop=True)
                nc.vector.tensor_tensor(lT[:, csl], lT[:, csl], ps[:], ALU.add)

        w1_tiles = {}
        eparams = {}

        def fetch_expert(e):
            if e >= E or e in w1_tiles:
                return
            strips = []
            for m in range(MH):
                t = wpool.tile([128, KD, 128], BF16, tag=f"w1_{m}",
                               name=f"w1_{e}_{m}")
                nc.sync.dma_start(t[:], w1p[e, :, m])
                strips.append(t)
            w2sb = epool.tile([128, MH, E], BF16, tag="w2", name=f"w2_{e}")
            nc.sync.dma_start(w2sb[:], w2p[e])
            b1sb = epool.tile([128, MH], F32, tag="b1", name=f"b1_{e}")
            nc.sync.dma_start(b1sb[:], b1p[e])
            w1_tiles[e] = strips
            eparams[e] = (w2sb, b1sb)

        w8_tiles = {}

        def emit_w8(e):
            w8b = w8p.tile([E, BS], BF16, tag="w8b", name=f"w8_{e}")
            for c in range(NCH):
                csl = slice(c * CHUNK, (c + 1) * CHUNK)
                w8 = psC.tile([E, CHUNK], F32, tag="tp")
                nc.tensor.matmul(w8[:], sel_sb[:, e, :], wT[:, csl],
                                 start=True, stop=True)
                nc.vector.tensor_copy(w8b[:, csl], w8[:])
            w8_tiles[e] = w8b

        def emit_drain(e, c, ps2):
            if e not in w8_tiles:
                emit_w8(e)
            csl = slice(c * CHUNK, (c + 1) * CHUNK)
            tmp = spool.tile([E, CHUNK], F32, tag="ltmp")
            nc.vector.tensor_tensor(tmp[:], ps2[:], w8_tiles[e][:, csl],
                                    ALU.mult)
            nc.vector.tensor_tensor(lT[:, csl], lT[:, csl], tmp[:], ALU.add)

        # ---------------- Phase A: first chunk of LN ----------------
        for bt in range(4):
            emit_ln_tile(bt)

        # ---------------- Main super-group stream ----------------
        # Super-group = (e, c, m2) covering m = 2*m2, 2*m2+1.  256 total,
        # ~3.7us each.  Side work is scheduled by super-group index.
        side_work = {}
        for i in range(12):                      # LN tiles 4..15: chain,
            side_work.setdefault(i, []).append(  # then transposes 1 sg later
                lambda bt=4 + i: emit_ln_chain(bt))
            side_work.setdefault(i + 1, []).append(
                lambda bt=4 + i: emit_ln_tp(bt))
        side_work.setdefault(7, []).append(emit_gate_mu)
        for i in range(16):                      # gate z DMAs (4 sg lead)
            side_work.setdefault(8 + i // 2, []).append(
                lambda bt=i: emit_z_dma(bt))
        for i in range(16):                      # z normalize chain
            side_work.setdefault(12 + i // 2, []).append(
                lambda bt=i: emit_gate_z(bt))
        for i in range(16):                      # z transposes 1 sg later
            side_work.setdefault(13 + i // 2, []).append(
                lambda bt=i: emit_z_tp(bt))
        for i in range(16):                      # sims + softmax, then wT
            side_work.setdefault(15 + i // 2, []).append(
                lambda bt=i: emit_gate_sims(bt))
            side_work.setdefault(16 + i // 2, []).append(
                lambda bt=i: emit_wT(bt))
        side_work.setdefault(25, []).append(emit_b2fold)
        side_work.setdefault(30, []).append(
            lambda: nc.sync.dma_start(
                w_o.rearrange("p (bo c) -> p bo c", c=E), w_sb[:]))

        def emit_ln_chain(bt):
            emit_ln_tile(bt, tp_inline=False)

        fetch_expert(0)
        sgroups = [(e, c, m2) for e in range(E) for c in range(NCH)
                   for m2 in range(MH // 2)]
        pend_h = []              # [(e, c, m, hsb), ...] awaiting mm2
        pend_drain = []          # [[countdown, e, c, ps2], ...]
        ps2_cur = None

        def flush_mm2(n, gi):
            nonlocal ps2_cur
            batch = pend_h[:n]
            del pend_h[:n]
            ms = [b[2] for b in batch]
            order = sorted(range(len(batch)), key=lambda i: -ms[i])
            if 0 in ms:                       # start must execute first
                i0 = ms.index(0)
                order.remove(i0)
                order.insert(0, i0)
            if MH - 1 in ms:                  # stop must execute last
                i15 = ms.index(MH - 1)
                order.remove(i15)
                order.append(i15)
            for i in order:
                pe, pc, pm, ph = batch[i]
                if pm == 0:
                    ps2_cur = psB.tile([E, CHUNK], F32, tag="ps2",
                                       name=f"ps2_{pe}_{pc}")
                pw2, _ = eparams[pe]
                nc.tensor.matmul(
                    ps2_cur[:], pw2[:, pm, :], ph[:],
                    start=(pm == 0), stop=(pm == MH - 1))
                if pm == MH - 1:
                    pend_drain.append([max(2, 27 - gi), pe, pc, ps2_cur])

        # Four dedicated ps1 banks.  Within a super-group the first m-half
        # writes the bank whose previous reader (relu) is NEWER, so its WAR
        # wait subsumes the second half's and the tile framework elides one
        # semaphore wait per super-group.
        SLOT = {0: (0, 1), 1: (2, 3), 2: (1, 0), 3: (3, 2)}

        def mm1_group(e, c, m, slot):
            strips = w1_tiles[e]
            _, b1sb = eparams[e]
            ps1 = psA.tile([128, CHUNK], F32, tag=f"ps1_{slot}")
            for k in range(KD):
                nc.tensor.matmul(
                    ps1[:], strips[m][:, k, :], xhatT_c[c][:, k, :],
                    start=(k == 0), stop=(k == KD - 1))
            hsb = hpool.tile([128, CHUNK], BF16, tag="h")
            nc.scalar.activation(
                hsb[:], ps1[:], AF.Relu, bias=b1sb[:, m:m + 1])
            pend_h.append((e, c, m, hsb))

        for gi, (e, c, m2) in enumerate(sgroups):
            if c == 0 and m2 == 1:
                fetch_expert(e + 1)
            sa, sb = SLOT[gi % 4]
            mm1_group(e, c, 2 * m2, sa)
            mm1_group(e, c, 2 * m2 + 1, sb)

            if len(pend_h) > 6:
                flush_mm2(4, gi)

            for item in pend_drain:
                item[0] -= 1
            while pend_drain and pend_drain[0][0] <= 0:
                _, de, dc, dps2 = pend_drain.pop(0)
                emit_drain(de, dc, dps2)

            for fn in side_work.pop(gi, ()):
                fn()

        # Tail: remaining mm2 batches + drains.
        while pend_h:
            flush_mm2(min(4, len(pend_h)), len(sgroups))
        for _, de, dc, dps2 in pend_drain:
            emit_drain(de, dc, dps2)

        # ---------------- Outputs (contiguous; host reorders) ----------
        nc.sync.dma_start(logits_o[:, :], lT[:])

    nc.compile()
    return nc


_CACHE = {}
_PREP = {}


def _prepare_params(W1, b1, W2, b2, ln_gamma, ln_beta):
    """Fold LN affine into W1/b1, convert + lay out for the kernel."""
    key = id(W1)
    if key in _PREP:
        return _PREP[key]
    if np.all(ln_gamma == 1.0):
        W1f = W1.astype(np.float32)
    else:
        W1f = (ln_gamma[:, :, None].astype(np.float64) *
               W1.astype(np.float64)).astype(np.float32)
    if np.all(ln_beta == 0.0):
        b1f = b1.astype(np.float32)
    else:
        b1f = (b1.astype(np.float64) +
               np.einsum('ed,edh->eh', ln_beta.astype(np.float64),
                         W1.astype(np.float64))).astype(np.float32)

    w1pp = np.ascontiguousarray(
        W1f.reshape(E, KD, 128, MH, 128).transpose(0, 2, 3, 1, 4)
    ).astype(ml_dtypes.bfloat16)
    w2pp = np.ascontiguousarray(
        W2.astype(np.float32).reshape(E, MH, 128, E).transpose(0, 2, 1, 3)
    ).astype(ml_dtypes.bfloat16)
    b1pp = np.ascontiguousarray(
        b1f.reshape(E, MH, 128).transpose(0, 2, 1))
    b2rp = np.ascontiguousarray(b2.astype(np.float32)).astype(
        ml_dtypes.bfloat16)
    selp = np.zeros((E, E, E), ml_dtypes.bfloat16)
    for e in range(E):
        selp[e, e, :] = 1.0
    _PREP.clear()
    _PREP[key] = (w1pp, b1pp, w2pp, b2rp, selp)
    return _PREP[key]


def kernel(**inputs):
    feat = np.ascontiguousarray(inputs["feat"], dtype=np.float32)
    z_cat = np.ascontiguousarray(inputs["z_cat"], dtype=np.float32)
    mu_cat = np.ascontiguousarray(inputs["mu_cat"], dtype=np.float32)
    ln_gamma = np.asarray(inputs["ln_gamma"], dtype=np.float32)
    ln_beta = np.asarray(inputs["ln_beta"], dtype=np.float32)
    tau = max(1e-6, float(inputs["tau_gate"]))

    w1pp, b1pp, w2pp, b2rp, selp = _prepare_params(
        np.asarray(inputs["W1"]), np.asarray(inputs["b1"]),
        np.asarray(inputs["W2"]), np.asarray(inputs["b2"]),
        ln_gamma, ln_beta)

    if tau not in _CACHE:
        _CACHE[tau] = _build(tau)
    nc = _CACHE[tau]

    in_maps = []
    for c in range(NCORES):
        rs = slice(c * BS, (c + 1) * BS)
        in_maps.append({
            "feat": feat[rs],
            "z": z_cat[rs],
            "mu": mu_cat,
            "w1p": w1pp,
            "b1p": b1pp,
            "w2p": w2pp,
            "b2r": b2rp,
            "selp": selp,
        })
    global LAST_IN_MAPS
    LAST_IN_MAPS = in_maps

    res = run_bass_kernel_spmd(nc, in_maps, core_ids=list(range(NCORES)))
    outs = res.results
    logits = np.concatenate([o["logits"].T for o in outs], axis=0)
    w = np.concatenate(
        [o["w"].reshape(128, BT, E).transpose(1, 0, 2).reshape(BS, E)
         for o in outs], axis=0)
    return logits.astype(np.float32), w.astype(np.float32)
